# revision 15
# baseline (speedup 1.0000x reference)
"""Batched multi-head attention (32 heads, S=2048, D=128, fp32) on 8 Trainium2
NeuronCores. Head-parallel sharding: core i computes heads [4i, 4i+4)
independently (no collectives), takes full fp32 inputs, returns the full fp32
output.

v2 design (vs the v1 181us kernel):
  - Q and K are pre-transposed to [h, d, s] on the HOST, so QT/KT load into
    SBUF with a plain dense cast-DMA (fp32->fp16). No PE-transposes, no DVE
    PSUM->SBUF copies, no psum slot borrowing.
  - q-chunks of 512 (4 per head). Scores^T for one (sk, chunk) is a
    [128, 512] fp32 psum block = exactly 1 bank. One big 6-bank psum tile
    holds a rolling window of 6 such blocks (sk-instance m -> bank m%6).
  - exp runs as ONE activation instruction per QUAD (4 sk tiles): a strided
    AP over 4 of the 6 banks ([128, 2048] = 4x fewer, 4x bigger ACT
    instructions; ACT per-instr overhead ~220-430ns amortizes). Quad bank
    sets cycle {0123} {4501} {2345}; the wrapped set uses a negative-stride
    AP so elements stay in natural sk order. While exp reads 4 banks, the
    PE fills the other 2 with the next quad's QK -> rolling, no stall.
  - PV: pt slice [128 sk, 128 q] stationary, V_aug [128 sk, 129] moving
    (col 128 = ones -> softmax denominator rides along). 16 MMs per quad
    accumulate into a 2-bank po tile: slices 0-2 at offsets 0/129/258
    (bank 6), slice 3 at offset 512 (bank 7). start=True only on the first
    MM touching each bank (clears the whole bank), stop at sk==15.
  - Normalize per chunk: 4x (DVE reciprocal of the ones column +
    tensor_scalar_mul) -> [128, 512] fp32 out tile, one DMA store.
  - Software-pipelined: PV(quad j-1) is emitted after exp(quad j); the
    chunk's normalize after the next chunk's first exp. po reuse across
    chunks is covered by the exp latency.
"""

import os
import numpy as np

BH, S, D = 32, 2048, 128
N_CORES = 8
HPC = BH // N_CORES  # heads per core
SK = S // 128        # 16 key tiles per head
CHUNK = 512          # q-chunk
NCH = S // CHUNK     # 4 chunks per head
QPC = SK // 4        # 4 quads per chunk
SCALE = 1.0 / float(np.sqrt(D))
PO_OFF = (0, 129, 258, 512)  # po column offsets for the 4 q-subtiles

_CACHE = {}


def _install_ntff_hook():
    """Provide antenv.axon_hooks (absent in this container) so that
    run_bass_kernel_spmd(trace=True) can capture NTFF profiles."""
    import contextlib, ctypes, sys, types

    if "antenv.axon_hooks" in sys.modules:
        return
    so_path = "/opt/axon/libaxon_pjrt.so"
    hook = None
    try:
        lib = ctypes.CDLL(so_path)
        if hasattr(lib, "axon_start_nrt_profile"):
            lib.axon_start_nrt_profile.argtypes = [
                ctypes.POINTER(ctypes.c_int64),
                ctypes.c_size_t,
            ]
            lib.axon_start_nrt_profile.restype = ctypes.c_int64
            lib.axon_stop_nrt_profile.argtypes = [ctypes.c_char_p]
            lib.axon_stop_nrt_profile.restype = ctypes.c_int64

            @contextlib.contextmanager
            def _h(output_dir, device_ids):
                import jax

                jax.devices()
                if device_ids:
                    ids = (ctypes.c_int64 * len(device_ids))(*device_ids)
                    rc = lib.axon_start_nrt_profile(ids, len(device_ids))
                else:
                    rc = lib.axon_start_nrt_profile(None, 0)
                if rc != 0:
                    raise RuntimeError(f"axon_start_nrt_profile rc={rc}")
                try:
                    yield
                finally:
                    n = lib.axon_stop_nrt_profile(str(output_dir).encode())
                    print(f"ntff profile: {n} file(s) in {output_dir}")

            hook = _h
    except OSError:
        pass
    mod = types.ModuleType("antenv.axon_hooks")
    mod.get_axon_ntff_profile_hook = lambda: hook
    mod.set_axon_ntff_profile_hook = lambda h: None
    sys.modules["antenv.axon_hooks"] = mod


def _split_sync_waits(nc, maxw=1):
    """The walrus codegen in this container rejects instructions carrying more
    than `maxw` sync waits (Tile's scheduler can attach several). Move the
    excess waits onto same-engine nop instructions inserted just before."""
    from concourse import mybir

    n_split = 0
    for f in nc.m.functions:
        for bb in f.blocks:
            out = []
            for inst in bb.instructions:
                si = inst.sync_info
                if si is not None and si.on_wait and len(si.on_wait) > maxw:
                    waits = list(si.on_wait)
                    carriers, keep = waits[:-maxw], waits[-maxw:]
                    si.on_wait = keep
                    inst.sync_info = si
                    for i in range(0, len(carriers), maxw):
                        n_split += 1
                        nop = mybir.InstNoOp(
                            name=f"{inst.name}_wsplit{i}", ins=[], outs=[]
                        )
                        nop.engine = inst.engine
                        nop.sync_info = mybir.SyncInfo(
                            on_wait=carriers[i : i + maxw], on_update=[]
                        )
                        if hasattr(nc, "inst_map"):
                            nc.inst_map[nop.name] = nop
                        out.append(nop)
                out.append(inst)
            bb.instructions[:] = out
    return n_split


def _build():
    # exact sub-tile range analysis (default caps dependency-scan work at
    # 100 and falls back to conservative engine-level deps, which serializes
    # the rolling scores window behind every exp)
    os.environ["TILE_EXHAUSTIVE_MEMORY_SHARE_CHECK"] = "1"
    import concourse.bass as bass
    from concourse import mybir
    import concourse.tile as tile
    import bass_rust

    fp16 = mybir.dt.float16
    fp32 = mybir.dt.float32
    AF = mybir.ActivationFunctionType

    from concourse.vector_clock import ScopedClock

    class SlimExitTileContext(tile.TileContext):
        def _drain_and_barrier(self, tick_clock, wait_clock):
            nc = self.nc
            drain_inst = nc.sync.drain()
            wait_clock.add_sem_waits(
                drain_inst.ins, ScopedClock({None: tick_clock.global_clock})
            )
            nc.all_engine_barrier()
            assert self.sems is not None
            popped = nc._tile_sem_poison_stack.pop()
            assert popped is self._sem_poison
            nc.clear_and_free_semaphores(list(self.sems.allocated().values()))
            nc.all_engine_barrier(sem_only=True)

    nc = bass.Bass("TRN2", target_bir_lowering=False, debug=False)
    # q, k arrive HOST-pre-transposed to [h, d, s]; v in natural [h, s, d]
    q = nc.dram_tensor("q", [HPC, D, S], fp32, kind="ExternalInput").ap()
    k = nc.dram_tensor("k", [HPC, D, S], fp32, kind="ExternalInput").ap()
    v = nc.dram_tensor("v", [HPC, S, D], fp32, kind="ExternalInput").ap()
    o = nc.dram_tensor("o", [HPC, S, D], fp32, kind="ExternalOutput").ap()

    with SlimExitTileContext(nc) as tc:
        with (
            tc.tile_pool(name="qt", bufs=2) as qt_pool,
            tc.tile_pool(name="kt", bufs=2) as kt_pool,
            tc.tile_pool(name="vsb", bufs=2) as v_pool,
            tc.tile_pool(name="pt", bufs=3) as pt_pool,
            tc.tile_pool(name="sc", bufs=2, space="PSUM") as sc_pool,
            tc.tile_pool(name="po", bufs=1, space="PSUM") as po_pool,
            tc.tile_pool(name="outsb", bufs=3) as out_pool,
            tc.tile_pool(name="norm", bufs=8) as norm_pool,
        ):


            qts, kts, vsbs, pos = {}, {}, {}, {}

            def prep_head(h, piece):
                """Load piece (0-3) of head h's inputs: kt quarter, qt
                quarter (chunk), v quarter. piece 0 also allocates tiles
                and sets the vsb ones columns (strided memset, 16 elems)."""
                if h >= HPC:
                    return
                if piece == 0:
                    qts[h] = qt_pool.tile([128, S], fp16, tag="qt", name=f"qt_{h}")
                    kts[h] = kt_pool.tile([128, S], fp16, tag="kt", name=f"kt_{h}")
                    vsbs[h] = v_pool.tile(
                        [128, SK * 129], fp16, tag="vsb", name=f"vsb_{h}"
                    )
                cs = slice(piece * 512, (piece + 1) * 512)
                nc.gpsimd.dma_start(kts[h][:, cs], k[h, :, cs])
                nc.gpsimd.dma_start(qts[h][:, cs], q[h, :, cs])
                vv = vsbs[h][:].rearrange("p (t c) -> p t c", c=129)
                if piece == 0:
                    nc.gpsimd.memset(vv[:, :, D : D + 1], 1.0)
                ts = slice(piece * 4, (piece + 1) * 4)
                rows = slice(piece * 512, (piece + 1) * 512)
                nc.gpsimd.dma_start(
                    vv[:, ts, 0:D], v[h, rows, :].rearrange("(t p) d -> p t d", p=128)
                )

            for piece in range(4):
                prep_head(0, piece)

            def emit_finalize(h, qc):
                """Normalize chunk (h, qc): one DVE copy drains po to SBUF
                (frees the po banks ASAP for the next chunk's PV), then
                reciprocal + scale from SBUF."""
                po, qbase = pos[(h, qc)], qc * CHUNK
                nb = norm_pool.tile(
                    [128, 2 * 387], fp32, tag="nb", name=f"nb_{h}_{qc}"
                )
                nc.vector.tensor_copy(
                    nb[:].rearrange("p (a c) -> p a c", c=387),
                    po[:].rearrange("p (a c) -> p a c", c=512)[:, :, 0:387],
                )
                ob = out_pool.tile(
                    [128, CHUNK], fp32, tag="ob", name=f"ob_{h}_{qc}"
                )
                for sq in range(4):
                    off = 129 * sq if sq < 3 else 387
                    r = norm_pool.tile(
                        [128, 1], fp32, tag="r", name=f"r_{h}_{qc}_{sq}"
                    )
                    nc.vector.reciprocal(r[:], nb[:, off + D : off + D + 1])
                    nc.vector.tensor_scalar_mul(
                        ob[:, sq * D : (sq + 1) * D],
                        nb[:, off : off + D],
                        r[:],
                    )
                nc.sync.dma_start(
                    o[h, qbase : qbase + CHUNK, :].rearrange(
                        "(t p) d -> p t d", p=128
                    ),
                    ob[:].rearrange("p (t d) -> p t d", d=D),
                )

            def emit_pv(ms, pt):
                """PV matmuls for the sk-instances `ms` of one exp group
                (pt column block i <-> ms[i]). Emits the chunk's normalize
                right after its last (sk==15) PV batch so the po slot
                rotation stays ordered."""
                for i, m in enumerate(ms):
                    h, qc, sk = m // 64, (m // 16) % 4, m % 16
                    if sk == 0:
                        pos[(h, qc)] = po_pool.tile(
                            [128, 1024], fp32, tag="po", name=f"po_{h}_{qc}"
                        )
                    po, vsb = pos[(h, qc)], vsbs[h]
                    for sq in range(4):
                        off = PO_OFF[sq]
                        nc.tensor.matmul(
                            po[:, off : off + 129],
                            pt[:, i * 512 + sq * 128 : i * 512 + sq * 128 + 128],
                            vsb[:, sk * 129 : (sk + 1) * 129],
                            start=(sk == 0 and (sq == 0 or sq == 3)),
                            stop=(sk == SK - 1),
                            skip_group_check=True,
                        )
                    if sk == SK - 1:
                        emit_finalize(h, qc)

            # triples of sk-instances; the ragged single goes FIRST so the
            # pipeline primes off one QK. A triple may span chunk/head
            # boundaries -- exp is elementwise, PV/normalize bookkeeping
            # handles the mapping per instance.
            M = HPC * NCH * SK
            groups = [[0]] + [list(range(g, g + 3)) for g in range(1, M, 3)]
            def emit_qk(ms):
                """QK matmuls for one group into a fresh ping-pong scores
                tile (pool bufs=2 -> WAR vs the exp two groups ago, tracked
                exactly by slot rotation). Returns the scores tile."""
                sct = sc_pool.tile([128, len(ms) * 512], fp32, tag="sc")
                for i, m in enumerate(ms):
                    h, qc, sk = m // 64, (m // 16) % 4, m % 16
                    if sk == 0:
                        prep_head(h + 1, qc)
                    nc.tensor.matmul(
                        sct[:, i * 512 : (i + 1) * 512],
                        kts[h][:, sk * 128 : (sk + 1) * 128],
                        qts[h][:, qc * CHUNK : qc * CHUNK + 512],
                        start=True,
                        stop=True,
                        skip_group_check=True,
                    )
                return sct

            # emission per group T: exp(T), QK(T+1), PV(T-1). Putting the
            # next group's QKs ahead of the PV batch keeps the exp chain fed
            # even when the PV batch stalls on a chunk-boundary po reuse.
            pending = None
            scts = {0: emit_qk(groups[0])}
            for T, ms in enumerate(groups):
                n = len(ms)
                pt = pt_pool.tile([128, n * 512], fp16, tag="pt")
                nc.scalar.activation(pt[:], scts.pop(T)[:], AF.Exp, scale=SCALE)
                if T + 1 < len(groups):
                    scts[T + 1] = emit_qk(groups[T + 1])
                if pending is not None:
                    pending()
                pending = lambda ms=ms, pt=pt: emit_pv(ms, pt)
            pending()

    _split_sync_waits(nc, maxw=1)
    return nc


def _get_nc():
    if "nc" not in _CACHE:
        _install_ntff_hook()
        _CACHE["nc"] = _build()
    return _CACHE["nc"]


def run_sharded(query, key, value, trace=False, **trace_kwargs):
    """Run the 8-core SPMD kernel; returns (output [BH,S,D] fp32, results)."""
    from concourse.bass_utils import run_bass_kernel_spmd

    nc = _get_nc()
    query = np.asarray(query, dtype=np.float32)
    key = np.asarray(key, dtype=np.float32)
    value = np.ascontiguousarray(np.asarray(value, dtype=np.float32))
    # host-side layout prep: Q, K as [h, d, s] for direct transposed loads
    qT = np.ascontiguousarray(query.transpose(0, 2, 1))
    kT = np.ascontiguousarray(key.transpose(0, 2, 1))
    in_maps = [
        {
            "q": qT[c * HPC : (c + 1) * HPC],
            "k": kT[c * HPC : (c + 1) * HPC],
            "v": value[c * HPC : (c + 1) * HPC],
        }
        for c in range(N_CORES)
    ]
    res = run_bass_kernel_spmd(
        nc, in_maps, list(range(N_CORES)), trace=trace, **trace_kwargs
    )
    out = np.concatenate([r["o"] for r in res.results], axis=0)
    return out, res


def kernel(key, query, value):
    out, _ = run_sharded(query, key, value, trace=False)
    return out


# revision 18
# speedup vs baseline: 1.2125x; 1.2125x over previous
"""Batched multi-head attention (32 heads, S=2048, D=128, fp32) on 8 Trainium2
NeuronCores. Head-parallel sharding: core i computes heads [4i, 4i+4)
independently (no collectives), takes full fp32 inputs, returns the full fp32
output.

v2 design (vs the v1 181us kernel):
  - Q and K are pre-transposed to [h, d, s] on the HOST, so QT/KT load into
    SBUF with a plain dense cast-DMA (fp32->fp16). No PE-transposes, no DVE
    PSUM->SBUF copies, no psum slot borrowing.
  - q-chunks of 512 (4 per head). Scores^T for one (sk, chunk) is a
    [128, 512] fp32 psum block = exactly 1 bank. One big 6-bank psum tile
    holds a rolling window of 6 such blocks (sk-instance m -> bank m%6).
  - exp runs as ONE activation instruction per QUAD (4 sk tiles): a strided
    AP over 4 of the 6 banks ([128, 2048] = 4x fewer, 4x bigger ACT
    instructions; ACT per-instr overhead ~220-430ns amortizes). Quad bank
    sets cycle {0123} {4501} {2345}; the wrapped set uses a negative-stride
    AP so elements stay in natural sk order. While exp reads 4 banks, the
    PE fills the other 2 with the next quad's QK -> rolling, no stall.
  - PV: pt slice [128 sk, 128 q] stationary, V_aug [128 sk, 129] moving
    (col 128 = ones -> softmax denominator rides along). 16 MMs per quad
    accumulate into a 2-bank po tile: slices 0-2 at offsets 0/129/258
    (bank 6), slice 3 at offset 512 (bank 7). start=True only on the first
    MM touching each bank (clears the whole bank), stop at sk==15.
  - Normalize per chunk: 4x (DVE reciprocal of the ones column +
    tensor_scalar_mul) -> [128, 512] fp32 out tile, one DMA store.
  - Software-pipelined: PV(quad j-1) is emitted after exp(quad j); the
    chunk's normalize after the next chunk's first exp. po reuse across
    chunks is covered by the exp latency.
"""

import os
import numpy as np

BH, S, D = 32, 2048, 128
N_CORES = 8
HPC = BH // N_CORES  # heads per core
SK = S // 128        # 16 key tiles per head
CHUNK = 512          # q-chunk
NCH = S // CHUNK     # 4 chunks per head
QPC = SK // 4        # 4 quads per chunk
SCALE = 1.0 / float(np.sqrt(D))
PO_OFF = (0, 129, 258, 512)  # po column offsets for the 4 q-subtiles

_CACHE = {}


def _install_ntff_hook():
    """Provide antenv.axon_hooks (absent in this container) so that
    run_bass_kernel_spmd(trace=True) can capture NTFF profiles."""
    import contextlib, ctypes, sys, types

    if "antenv.axon_hooks" in sys.modules:
        return
    so_path = "/opt/axon/libaxon_pjrt.so"
    hook = None
    try:
        lib = ctypes.CDLL(so_path)
        if hasattr(lib, "axon_start_nrt_profile"):
            lib.axon_start_nrt_profile.argtypes = [
                ctypes.POINTER(ctypes.c_int64),
                ctypes.c_size_t,
            ]
            lib.axon_start_nrt_profile.restype = ctypes.c_int64
            lib.axon_stop_nrt_profile.argtypes = [ctypes.c_char_p]
            lib.axon_stop_nrt_profile.restype = ctypes.c_int64

            @contextlib.contextmanager
            def _h(output_dir, device_ids):
                import jax

                jax.devices()
                if device_ids:
                    ids = (ctypes.c_int64 * len(device_ids))(*device_ids)
                    rc = lib.axon_start_nrt_profile(ids, len(device_ids))
                else:
                    rc = lib.axon_start_nrt_profile(None, 0)
                if rc != 0:
                    raise RuntimeError(f"axon_start_nrt_profile rc={rc}")
                try:
                    yield
                finally:
                    n = lib.axon_stop_nrt_profile(str(output_dir).encode())
                    print(f"ntff profile: {n} file(s) in {output_dir}")

            hook = _h
    except OSError:
        pass
    mod = types.ModuleType("antenv.axon_hooks")
    mod.get_axon_ntff_profile_hook = lambda: hook
    mod.set_axon_ntff_profile_hook = lambda h: None
    sys.modules["antenv.axon_hooks"] = mod


def _split_sync_waits(nc, maxw=1):
    """The walrus codegen in this container rejects instructions carrying more
    than `maxw` sync waits (Tile's scheduler can attach several). Move the
    excess waits onto same-engine nop instructions inserted just before."""
    from concourse import mybir

    n_split = 0
    for f in nc.m.functions:
        for bb in f.blocks:
            out = []
            for inst in bb.instructions:
                si = inst.sync_info
                if si is not None and si.on_wait and len(si.on_wait) > maxw:
                    waits = list(si.on_wait)
                    carriers, keep = waits[:-maxw], waits[-maxw:]
                    si.on_wait = keep
                    inst.sync_info = si
                    for i in range(0, len(carriers), maxw):
                        n_split += 1
                        nop = mybir.InstNoOp(
                            name=f"{inst.name}_wsplit{i}", ins=[], outs=[]
                        )
                        nop.engine = inst.engine
                        nop.sync_info = mybir.SyncInfo(
                            on_wait=carriers[i : i + maxw], on_update=[]
                        )
                        if hasattr(nc, "inst_map"):
                            nc.inst_map[nop.name] = nop
                        out.append(nop)
                out.append(inst)
            bb.instructions[:] = out
    return n_split


def _build():
    # exact sub-tile range analysis (default caps dependency-scan work at
    # 100 and falls back to conservative engine-level deps, which serializes
    # the rolling scores window behind every exp)
    os.environ["TILE_EXHAUSTIVE_MEMORY_SHARE_CHECK"] = "1"
    import concourse.bass as bass
    from concourse import mybir
    import concourse.tile as tile
    import bass_rust

    fp16 = mybir.dt.float16
    fp32 = mybir.dt.float32
    AF = mybir.ActivationFunctionType

    from concourse.vector_clock import ScopedClock

    class SlimExitTileContext(tile.TileContext):
        def _drain_and_barrier(self, tick_clock, wait_clock):
            nc = self.nc
            drain_inst = nc.sync.drain()
            wait_clock.add_sem_waits(
                drain_inst.ins, ScopedClock({None: tick_clock.global_clock})
            )
            nc.all_engine_barrier()
            assert self.sems is not None
            popped = nc._tile_sem_poison_stack.pop()
            assert popped is self._sem_poison
            nc.clear_and_free_semaphores(list(self.sems.allocated().values()))
            nc.all_engine_barrier(sem_only=True)

    nc = bass.Bass("TRN2", target_bir_lowering=False, debug=False)
    # q, k arrive HOST-pre-transposed to [h, d, s] and HOST-cast to fp16;
    # v is host-cast fp16 in natural [h, s, d]. All input DMAs are then
    # plain HWDGE loads (hardware descriptor gen -- no Q7 serialization).
    q = nc.dram_tensor("q", [HPC, D, S], fp16, kind="ExternalInput").ap()
    k = nc.dram_tensor("k", [HPC, D, S], fp16, kind="ExternalInput").ap()
    v = nc.dram_tensor("v", [HPC, S, D], fp16, kind="ExternalInput").ap()
    o = nc.dram_tensor("o", [HPC, S, D], fp32, kind="ExternalOutput").ap()

    with SlimExitTileContext(nc) as tc:
        with (
            tc.tile_pool(name="qt", bufs=2) as qt_pool,
            tc.tile_pool(name="kt", bufs=2) as kt_pool,
            tc.tile_pool(name="vsb", bufs=2) as v_pool,
            tc.tile_pool(name="pt", bufs=3) as pt_pool,
            tc.tile_pool(name="sc", bufs=2, space="PSUM") as sc_pool,
            tc.tile_pool(name="po", bufs=1, space="PSUM") as po_pool,
            tc.tile_pool(name="outsb", bufs=3) as out_pool,
            tc.tile_pool(name="norm", bufs=8) as norm_pool,
        ):


            qts, kts, vsbs, pos = {}, {}, {}, {}

            def prep_head(h, piece):
                """Load piece (0-3) of head h's inputs: kt quarter, qt
                quarter (chunk), v quarter. piece 0 also allocates tiles
                and sets the vsb ones columns (strided memset, 16 elems)."""
                if h >= HPC:
                    return
                if piece == 0:
                    qts[h] = qt_pool.tile([128, S], fp16, tag="qt", name=f"qt_{h}")
                    kts[h] = kt_pool.tile([128, S], fp16, tag="kt", name=f"kt_{h}")
                    vsbs[h] = v_pool.tile(
                        [128, SK * 129], fp16, tag="vsb", name=f"vsb_{h}"
                    )
                cs = slice(piece * 512, (piece + 1) * 512)
                nc.sync.dma_start(kts[h][:, cs], k[h, :, cs])
                nc.sync.dma_start(qts[h][:, cs], q[h, :, cs])
                vv = vsbs[h][:].rearrange("p (t c) -> p t c", c=129)
                if piece == 0:
                    nc.gpsimd.memset(vv[:, :, D : D + 1], 1.0)
                ts = slice(piece * 4, (piece + 1) * 4)
                rows = slice(piece * 512, (piece + 1) * 512)
                nc.sync.dma_start(
                    vv[:, ts, 0:D], v[h, rows, :].rearrange("(t p) d -> p t d", p=128)
                )

            for piece in range(4):
                prep_head(0, piece)

            def emit_finalize(h, qc):
                """Normalize chunk (h, qc): one DVE copy drains po to SBUF
                (frees the po banks ASAP for the next chunk's PV), then
                reciprocal + scale from SBUF."""
                po, qbase = pos[(h, qc)], qc * CHUNK
                nb = norm_pool.tile(
                    [128, 2 * 387], fp32, tag="nb", name=f"nb_{h}_{qc}"
                )
                nc.vector.tensor_copy(
                    nb[:].rearrange("p (a c) -> p a c", c=387),
                    po[:].rearrange("p (a c) -> p a c", c=512)[:, :, 0:387],
                )
                ob = out_pool.tile(
                    [128, CHUNK], fp32, tag="ob", name=f"ob_{h}_{qc}"
                )
                for sq in range(4):
                    off = 129 * sq if sq < 3 else 387
                    r = norm_pool.tile(
                        [128, 1], fp32, tag="r", name=f"r_{h}_{qc}_{sq}"
                    )
                    nc.vector.reciprocal(r[:], nb[:, off + D : off + D + 1])
                    nc.vector.tensor_scalar_mul(
                        ob[:, sq * D : (sq + 1) * D],
                        nb[:, off : off + D],
                        r[:],
                    )
                nc.sync.dma_start(
                    o[h, qbase : qbase + CHUNK, :].rearrange(
                        "(t p) d -> p t d", p=128
                    ),
                    ob[:].rearrange("p (t d) -> p t d", d=D),
                )

            def emit_pv(ms, pt):
                """PV matmuls for the sk-instances `ms` of one exp group
                (pt column block i <-> ms[i]). Emits the chunk's normalize
                right after its last (sk==15) PV batch so the po slot
                rotation stays ordered."""
                for i, m in enumerate(ms):
                    h, qc, sk = m // 64, (m // 16) % 4, m % 16
                    if sk == 0:
                        pos[(h, qc)] = po_pool.tile(
                            [128, 1024], fp32, tag="po", name=f"po_{h}_{qc}"
                        )
                    po, vsb = pos[(h, qc)], vsbs[h]
                    for sq in range(4):
                        off = PO_OFF[sq]
                        nc.tensor.matmul(
                            po[:, off : off + 129],
                            pt[:, i * 512 + sq * 128 : i * 512 + sq * 128 + 128],
                            vsb[:, sk * 129 : (sk + 1) * 129],
                            start=(sk == 0 and (sq == 0 or sq == 3)),
                            stop=(sk == SK - 1),
                            skip_group_check=True,
                        )
                    if sk == SK - 1:
                        emit_finalize(h, qc)

            # triples of sk-instances; the ragged single goes FIRST so the
            # pipeline primes off one QK. A triple may span chunk/head
            # boundaries -- exp is elementwise, PV/normalize bookkeeping
            # handles the mapping per instance.
            M = HPC * NCH * SK
            groups = [[0]] + [list(range(g, g + 3)) for g in range(1, M, 3)]
            def emit_qk(ms):
                """QK matmuls for one group into a fresh ping-pong scores
                tile (pool bufs=2 -> WAR vs the exp two groups ago, tracked
                exactly by slot rotation). Returns the scores tile."""
                sct = sc_pool.tile([128, len(ms) * 512], fp32, tag="sc")
                for i, m in enumerate(ms):
                    h, qc, sk = m // 64, (m // 16) % 4, m % 16
                    if sk == 0:
                        prep_head(h + 1, qc)
                    nc.tensor.matmul(
                        sct[:, i * 512 : (i + 1) * 512],
                        kts[h][:, sk * 128 : (sk + 1) * 128],
                        qts[h][:, qc * CHUNK : qc * CHUNK + 512],
                        start=True,
                        stop=True,
                        skip_group_check=True,
                    )
                return sct

            # emission per group T: exp(T), QK(T+1), PV(T-1). Putting the
            # next group's QKs ahead of the PV batch keeps the exp chain fed
            # even when the PV batch stalls on a chunk-boundary po reuse.
            pending = None
            scts = {0: emit_qk(groups[0])}
            for T, ms in enumerate(groups):
                n = len(ms)
                pt = pt_pool.tile([128, n * 512], fp16, tag="pt")
                nc.scalar.activation(pt[:], scts.pop(T)[:], AF.Exp, scale=SCALE)
                if T + 1 < len(groups):
                    scts[T + 1] = emit_qk(groups[T + 1])
                if pending is not None:
                    pending()
                pending = lambda ms=ms, pt=pt: emit_pv(ms, pt)
            pending()

    _split_sync_waits(nc, maxw=1)
    return nc


def _get_nc():
    if "nc" not in _CACHE:
        _install_ntff_hook()
        _CACHE["nc"] = _build()
    return _CACHE["nc"]


def run_sharded(query, key, value, trace=False, **trace_kwargs):
    """Run the 8-core SPMD kernel; returns (output [BH,S,D] fp32, results)."""
    from concourse.bass_utils import run_bass_kernel_spmd

    nc = _get_nc()
    query = np.asarray(query, dtype=np.float32)
    key = np.asarray(key, dtype=np.float32)
    value = np.ascontiguousarray(np.asarray(value, dtype=np.float16))
    # host-side layout prep: Q, K as [h, d, s] fp16 for direct transposed
    # loads (the kernel computes in fp16 with fp32 PSUM accumulate)
    qT = np.ascontiguousarray(query.transpose(0, 2, 1).astype(np.float16))
    kT = np.ascontiguousarray(key.transpose(0, 2, 1).astype(np.float16))
    in_maps = [
        {
            "q": qT[c * HPC : (c + 1) * HPC],
            "k": kT[c * HPC : (c + 1) * HPC],
            "v": value[c * HPC : (c + 1) * HPC],
        }
        for c in range(N_CORES)
    ]
    res = run_bass_kernel_spmd(
        nc, in_maps, list(range(N_CORES)), trace=trace, **trace_kwargs
    )
    out = np.concatenate([r["o"] for r in res.results], axis=0)
    return out, res


def kernel(key, query, value):
    out, _ = run_sharded(query, key, value, trace=False)
    return out
